# revision 36
# baseline (speedup 1.0000x reference)
"""Trainium2 Bass kernel for edge-softmax attention aggregation (GNN message passing).

Strategy: destination-sharded segment softmax (no cross-core collectives).
  - Host: snake-deal nodes (by degree) into 8 cores x 50 blocks x 4 subblocks
    of 32 node slots, so every subblock receives ~1000 edges; permute edges so
    each subblock owns a contiguous 128*c_sub-slot padded run (partition-major
    interleaved for contiguous per-partition DMA). cutoff/sqrt(dk) is folded
    into q; q|k are packed d-major (col = d*H + h) so the device head-sum
    tree is contiguous fp16 (DVE 2x mode); v is h-major. All fp16, 192 cols.
  - Device (per core, SPMD): per 4-subblock batch stream [128, 32, 192] fp16.
      DVE: qk mult, 3-level contiguous add tree -> w[128,s,8] (all fp16 2x),
           es*v mult, one-hot is_equal in transposed [128, node, chunk]
           layout against a materialized iota (2x; matmul reads the moving
           operand with stride cs2).
      ACT: es = exp(w - 2) head-replicated + 8-col denominator copy, and the
           per-block psum -> SBUF fp16 copy.
      PE:  scatter-add psum[:, strip] += [es*v | es].T @ onehot per chunk.
    The [72, 128] psum block (numerator rows 0:64, denominator rows 64:72) is
    copied to SBUF as fp16 and DMA'd out raw, 8 blocks per transfer; the
    final divide happens on the host during unshard. No transpose, no
    reciprocal, no gpsimd - the tail is just ACT copy + DMA.
  - Host: divide, inverse-permute rows to node order, zero degree-0 nodes.
"""

import sys

if "/opt/trn_rl_repo" not in sys.path:
    sys.path.insert(0, "/opt/trn_rl_repo")

import numpy as np

import concourse.bacc as bacc
import concourse.mybir as mybir
import concourse.tile as tile
from concourse.bass_utils import run_bass_kernel_spmd

F32 = mybir.dt.float32

N_NODES = 50000
N_EDGES = 1_600_000
DK = 64
H = 8
DH = 8  # per-head dim
NC = 8  # cores

SUB_NODES = 32      # node slots per subblock (= one-hot width = matmul M)
SUBS_PER_BLOCK = 4  # PSUM col strips per 128-node block
DEFAULT_BLOCKS = 50  # 128-node blocks per core


F32R = mybir.dt.float32r
BF16 = mybir.dt.bfloat16
FP16 = mybir.dt.float16
GPB = 4  # subblock groups batched per DMA / DVE op (must divide SUBS_PER_BLOCK)
OUT_GRP = 8  # blocks per output DMA


def build_program(c_sub: int, blocks: int, n_cores: int):
    """Build + compile the SPMD Bass program (one program, all cores)."""
    g_core = blocks * SUBS_PER_BLOCK        # subblock groups per core
    slots_sub = 128 * c_sub                 # edge slots per subblock
    cs2 = GPB * c_sub                       # chunks per batch

    nc = bacc.Bacc("TRN2", target_bir_lowering=False, debug=False,
                   num_devices=n_cores)
    qkv = nc.declare_dram_parameter(
        "qkv", [g_core * slots_sub, 3 * DK], FP16, isOutput=False)
    lidx = nc.declare_dram_parameter(
        "lidx", [128, g_core * c_sub], FP16, isOutput=False)
    iota = nc.declare_dram_parameter("iota", [128, SUB_NODES * cs2], FP16,
                                     isOutput=False)
    out = nc.declare_dram_parameter("out", [DK + 8, blocks * 128], FP16,
                                    isOutput=True)

    with tile.TileContext(nc) as tc, nc.allow_low_precision(
            "fp16 head-sum/es pipeline, ~5e-4 rel err vs fp32 reference"):
        with (
            tc.tile_pool(name="const", bufs=1) as cpool,
            tc.tile_pool(name="io", bufs=4) as iopool,
            tc.tile_pool(name="work", bufs=4) as wpool,
            tc.tile_pool(name="psA", bufs=4, space="PSUM") as ppA,
            tc.tile_pool(name="outp", bufs=3) as opool,
        ):
            # iota_cs[p, j, s] = j (materialized so the one-hot compare has a
            # packed inner dim on every operand -> DVE 2x mode).
            # Constants load from the gpsimd ring so the first qkv transfer
            # (sync ring) completes as early as possible.
            iota_t = cpool.tile([128, SUB_NODES, cs2], FP16)
            nc.gpsimd.dma_start(
                iota_t[:], iota[:].rearrange("p (j s) -> p j s", s=cs2))
            lidx_t = cpool.tile([128, g_core * c_sub], FP16)
            nc.gpsimd.dma_start(lidx_t[:], lidx[:])
            nbias = cpool.tile([128, 1], F32)
            nc.vector.memset(nbias[:], -2.0)

            ob = None
            dt2 = None
            for gb in range(g_core // GPB):   # one 128-node block per batch
                psum_t = ppA.tile([DK + 8, 128], F32, name="psum", tag="psum")

                # issue input DMA only from queues with no dependent compute
                # (sync + the otherwise-empty gpsimd queue); issuing from the
                # scalar queue couples DMA issue to ACT's dependency waits.
                # Two batches share one transfer: fewer ring boundaries keeps
                # the 16 DMA engines busier during the steady state.
                if gb % 2 == 0:
                    dt2 = iopool.tile([128, 2, cs2, 3 * DK], FP16)
                    # first transfers on the sync ring so batch 0 lands early
                    # (a cold burst across both rings delays its completion)
                    dma_eng = nc.sync if (gb < 4 or gb % 4 == 0) else nc.gpsimd
                    dma_eng.dma_start(
                        dt2[:],
                        qkv[gb * GPB * slots_sub:(gb + 2) * GPB * slots_sub, :]
                        .rearrange("(t p s) d -> p t s d", p=128, t=2),
                    )
                dt = dt2[:, gb % 2]

                # per-edge, per-head logits: q/k are stored d-major on the
                # host (col = d*H + h), so the whole reduction tree is
                # contiguous fp16 -> every DVE op runs in 2x mode.
                # Everything is processed in chunk halves so each half's
                # mult -> tree -> exp -> es*v chain runs independently and
                # the rhs tile completes earlier (shorter per-batch chain).
                qk = wpool.tile([128, cs2, DK], FP16)
                t1 = wpool.tile([128, cs2, 32], FP16)
                t2 = wpool.tile([128, cs2, 16], FP16)
                w = wpool.tile([128, cs2, H], FP16)
                rhs = wpool.tile([128, cs2, DK + 8], FP16)
                esr = wpool.tile([128, cs2, H, DH], FP16)
                ch = cs2 // 2
                for hf in range(2):
                    sl = slice(hf * ch, (hf + 1) * ch)
                    nc.vector.tensor_tensor(
                        qk[:, sl], dt[:, sl, 0:64], dt[:, sl, 64:128],
                        op=mybir.AluOpType.mult)
                    nc.vector.tensor_tensor(
                        t1[:, sl], qk[:, sl, 0:32], qk[:, sl, 32:64],
                        op=mybir.AluOpType.add)
                    nc.vector.tensor_tensor(
                        t2[:, sl], t1[:, sl, 0:16], t1[:, sl, 16:32],
                        op=mybir.AluOpType.add)
                    nc.vector.tensor_tensor(
                        w[:, sl], t2[:, sl, 0:8], t2[:, sl, 8:16],
                        op=mybir.AluOpType.add)
                    nc.scalar.activation(rhs[:, sl, 64:72], w[:, sl],
                                         mybir.ActivationFunctionType.Exp,
                                         bias=nbias[:])
                    nc.scalar.activation(
                        esr[:, sl],
                        w[:, sl].rearrange("p s (h o) -> p s h o", o=1)
                        .to_broadcast([128, ch, H, DH]),
                        mybir.ActivationFunctionType.Exp, bias=nbias[:])
                    nc.vector.tensor_tensor(
                        rhs[:, sl, 0:64]
                        .rearrange("p s (h d) -> p s h d", d=DH),
                        dt[:, sl, 128:192]
                        .rearrange("p s (h d) -> p s h d", d=DH),
                        esr[:, sl],
                        op=mybir.AluOpType.mult)

                # one-hot, transposed layout [128, node, chunk] so every
                # operand has a packed inner dim (DVE 2x); the matmul reads
                # its moving operand with stride cs2
                oh = wpool.tile([128, SUB_NODES, cs2], FP16)
                nc.vector.tensor_tensor(
                    oh[:],
                    lidx_t[:, gb * cs2:(gb + 1) * cs2]
                    .rearrange("p (o s) -> p o s", o=1)
                    .to_broadcast([128, SUB_NODES, cs2]),
                    iota_t[:],
                    op=mybir.AluOpType.is_equal)

                # scatter-add: psum[:, strip_j] += rhs.T @ onehot
                # (stationary = edge features, moving = one-hot)
                for s in range(cs2):
                    j = s // c_sub
                    nc.tensor.matmul(
                        psum_t[:, 32 * j:32 * (j + 1)],
                        lhsT=rhs[:, s, :], rhs=oh[:, :, s],
                        start=(s % c_sub == 0), stop=(s % c_sub == c_sub - 1))

                # block tail: copy raw [72, 128] psum (numerator | denominator)
                # to SBUF as fp16; DMA out 8 blocks per transfer from the
                # scalar queue (in-order after the copy -> no extra waits).
                # The divide happens on the host.
                b = gb
                if b % OUT_GRP == 0:
                    ob = opool.tile([DK + 8, OUT_GRP, 128], FP16)
                nc.scalar.copy(ob[:, b % OUT_GRP, :], psum_t[:])
                if b % OUT_GRP == OUT_GRP - 1 or b == blocks - 1:
                    b0 = (b // OUT_GRP) * OUT_GRP
                    nb = b - b0 + 1
                    nc.scalar.dma_start(
                        out[:, b0 * 128:(b + 1) * 128],
                        ob[:, 0:nb, :].rearrange("p o d -> p (o d)"))

    nc.compile()
    return nc


def prepare(key, value, query, edge_weight_cutoff, edge_index,
            blocks=DEFAULT_BLOCKS, n_cores=NC):
    """Host-side sharding: node->slot assignment, edge permutation, packing."""
    n_nodes = N_NODES
    n_edges = edge_index.shape[1]
    nsb = n_cores * blocks * SUBS_PER_BLOCK  # total subblocks

    dst = np.asarray(edge_index[1], dtype=np.int64)
    deg = np.bincount(dst, minlength=n_nodes)

    # snake-deal nodes (sorted by degree desc) into nsb bins -> balanced edges
    order_nodes = np.argsort(-deg, kind="stable")
    rounds = -(-n_nodes // nsb)
    assert rounds <= SUB_NODES, "too few subblocks for node count"
    padded = np.full(rounds * nsb, -1, dtype=np.int64)
    padded[:n_nodes] = order_nodes
    arr = padded.reshape(rounds, nsb)
    arr[1::2] = arr[1::2, ::-1]  # snake
    bin_of_node = np.empty(n_nodes, dtype=np.int64)
    slot_of_node = np.empty(n_nodes, dtype=np.int64)
    rr, cc = np.divmod(np.arange(rounds * nsb), nsb)
    flat = arr.reshape(-1)
    mask = flat >= 0
    bin_of_node[flat[mask]] = cc[mask]
    slot_of_node[flat[mask]] = rr[mask]

    bin_edges = np.bincount(bin_of_node[dst], minlength=nsb)
    c_sub = max(1, int(-(-bin_edges.max() // 128)))
    slots_sub = 128 * c_sub

    # group edges by subblock, pad each subblock to slots_sub
    sb_of_edge = bin_of_node[dst]
    eorder = np.argsort(sb_of_edge, kind="stable")
    counts = np.bincount(sb_of_edge, minlength=nsb)
    offsets = np.zeros(nsb + 1, dtype=np.int64)
    np.cumsum(counts, out=offsets[1:])
    sb_sorted = sb_of_edge[eorder]
    rank = np.arange(n_edges, dtype=np.int64) - offsets[sb_sorted]
    # position within the GPB-subblock DMA batch: partition-major interleave
    # so each 128-edge chunk stays subblock-pure under the (p s) device AP
    pp = rank // c_sub
    ss = rank % c_sub
    pos = ((sb_sorted // GPB) * (GPB * slots_sub) + pp * (GPB * c_sub)
           + (sb_sorted % GPB) * c_sub + ss)

    perm = np.full(nsb * slots_sub, n_edges, dtype=np.int64)
    perm[pos] = eorder
    lidx_flat = np.full(nsb * slots_sub, float(SUB_NODES + 7), dtype=np.float16)
    lidx_flat[pos] = slot_of_node[dst[eorder]].astype(np.float16)

    # pack q*cutoff/sqrt(dh) | k (both d-major: col = d*H + h, so the device
    # head-sum tree is contiguous) and v (h-major), all fp16, zero pad row
    scale = (np.asarray(edge_weight_cutoff, np.float32)
             * np.float32(1.0 / np.sqrt(DH)))
    dmaj = (np.arange(DK).reshape(H, DH).T.reshape(-1))  # dmaj[d*H+h] = h*DH+d
    packed = np.empty((n_edges + 1, 192), dtype=np.float16)
    packed[:n_edges, 0:64] = (np.asarray(query, np.float32)[:, dmaj]
                              * scale[:, None]).astype(np.float16)
    packed[:n_edges, 64:128] = np.asarray(key, np.float16)[:, dmaj]
    packed[:n_edges, 128:192] = np.asarray(value, np.float16)
    packed[n_edges] = 0.0

    g_core = blocks * SUBS_PER_BLOCK
    qkv_dev = packed[perm].reshape(n_cores, g_core * slots_sub, 192)
    lidx_dev = (lidx_flat.reshape(n_cores, g_core // GPB, 128, GPB * c_sub)
                .transpose(0, 2, 1, 3).reshape(n_cores, 128, g_core * c_sub))
    lidx_dev = np.ascontiguousarray(lidx_dev)
    # iota_cs[p, j*cs2 + s] = j (node-slot index, constant along chunk dim)
    cs2 = GPB * c_sub
    iota_np = np.tile(
        np.repeat(np.arange(SUB_NODES, dtype=np.float16), cs2), (128, 1))

    meta = dict(bin_of_node=bin_of_node, slot_of_node=slot_of_node, deg=deg,
                c_sub=c_sub, blocks=blocks, n_cores=n_cores)
    in_maps = [
        {"qkv": qkv_dev[c], "lidx": lidx_dev[c], "iota": iota_np}
        for c in range(n_cores)
    ]
    return in_maps, meta


def unshard(results, meta):
    """Host: divide numerator by denominator, back to [N_NODES, DK] order."""
    n_cores = meta["n_cores"]
    blocks = meta["blocks"]
    g_core = blocks * SUBS_PER_BLOCK
    # device out is [72, blocks*128] fp16: feature rows f for node col
    # (b*128 + n); numerator rows 0:64 (f = h*8+d), denominator rows 64+h
    allout = []
    for c in range(n_cores):
        o = np.asarray(results[c]["out"]).astype(np.float32)
        num = o[0:64, :]                     # [64, blocks*128]
        den = o[64:72, :]                    # [8, blocks*128]
        with np.errstate(divide="ignore", invalid="ignore"):
            res = num / np.repeat(den, DH, axis=0)   # [64, blocks*128]
        allout.append(res.T)                 # [blocks*128, 64]
    allout = np.stack(allout)

    bin_of_node = meta["bin_of_node"]
    slot_of_node = meta["slot_of_node"]
    core = bin_of_node // g_core
    g = bin_of_node % g_core
    row = (g // SUBS_PER_BLOCK) * 128 + (g % SUBS_PER_BLOCK) * 32 + slot_of_node
    out_full = allout[core, row]
    out_full[meta["deg"] == 0] = 0.0
    return out_full


_program_cache = {}


def kernel(key, value, query, edge_weight_cutoff, edge_index):
    in_maps, meta = prepare(key, value, query, edge_weight_cutoff, edge_index)
    cache_key = (meta["c_sub"], meta["blocks"], meta["n_cores"])
    if cache_key not in _program_cache:
        _program_cache[cache_key] = build_program(*cache_key)
    nc = _program_cache[cache_key]
    res = run_bass_kernel_spmd(nc, in_maps, list(range(meta["n_cores"])))
    return unshard(res.results, meta)
